# revision 16
# baseline (speedup 1.0000x reference)
"""CrossAttention Trainium2 kernel, SPMD over 8 NeuronCores.

Problem: x[4,2048,1024], context[4,1024,768], Wq[1024,512], Wk/Wv[768,512],
Wout[512,1024], bout[1024] -> out[4,2048,1024] (f32).

Sharding: 8 cores = 4 batches x 2 halves of the query dim n (2048 -> 2x1024).
Each core computes full attention for its (batch, n-half) with no collectives.

v6 design notes (trace-driven, evolved from v5):
- Startup is device-HBM-bound (8 cores x 7MB saturate HBM ~20us). Wq/Wk
  are stored pair-major so the 448KB the first sims need lands first;
  first exp fires ~14us instead of ~28us.
- ScalarE exp paces each phase (8 x ~1.08us). attnv for the previously
  simmed block + projection chains + the batched normalize chain are
  spread across slots so the PE never drains at phase boundaries (HAM
  stays at K=8/8).
- Batched normalize: both denominators -> SBUF bounce (custom-DVE recip
  reads garbage from PSUM), one [1,2,FB] reciprocal, one bf16 cast, two
  concurrent K=1 col-group broadcast MMs, one evacuation, two muls.
- Every attnv+normalize completes inside its phase, so the pp_ob psum
  rotation never blocks the next phase.
- Epilogue: outproj chains interleave with the last attnv block; all
  bias-adds ride the otherwise-idle ScalarE; dummy warm MMs keep HAM
  hot through pair-3 phase B.
"""

import numpy as np
import ml_dtypes

import concourse.bass as bass
import concourse.mybir as mybir
import concourse.tile as tile
from concourse import bacc
from concourse.bass_utils import run_bass_kernel_spmd

BF16 = mybir.dt.bfloat16
F32 = mybir.dt.float32

B, N, QD = 4, 2048, 1024
M, CD = 1024, 768
H, D = 8, 64
INNER = H * D  # 512
NSH = N // 2  # 1024 query rows per core
P = 128
FB = 512  # psum bank = 512 f32

KQ = QD // P  # 8 contraction tiles for q-proj
KC = CD // P  # 6 contraction tiles for k/v-proj
MI = INNER // P  # 4 head pairs
NB = NSH // FB  # 2 n blocks
MC = M // P  # 8 m chunks
KO = INNER // P  # 4 contraction tiles for out-proj
QT = QD // P  # 8 out-proj row tiles
MH = M // 2  # 512, ctxT descriptor split
KQH = KQ // 2


def build_nc():
    nc = bacc.Bacc(None)

    Wq0_d = nc.declare_dram_parameter("Wq0", [P, 1, KQ, P], BF16, isOutput=False)
    Wqr_d = nc.declare_dram_parameter("Wqr", [P, MI - 1, KQ, P], BF16, isOutput=False)
    xT0a_d = nc.declare_dram_parameter("xT0a", [P, KQH, FB], BF16, isOutput=False)
    xT0b_d = nc.declare_dram_parameter("xT0b", [P, KQH, FB], BF16, isOutput=False)
    Wk0_d = nc.declare_dram_parameter("Wk0", [P, 1, KC, P], BF16, isOutput=False)
    Wkr_d = nc.declare_dram_parameter("Wkr", [P, MI - 1, KC, P], BF16, isOutput=False)
    ctxTa_d = nc.declare_dram_parameter("ctxTa", [P, KC, MH], BF16, isOutput=False)
    ctxTb_d = nc.declare_dram_parameter("ctxTb", [P, KC, MH], BF16, isOutput=False)
    Wv_d = nc.declare_dram_parameter("Wv", [P, KC, INNER], BF16, isOutput=False)
    xT1_d = nc.declare_dram_parameter("xT1", [P, KQ, FB], BF16, isOutput=False)
    Wout_d = nc.declare_dram_parameter("Wout", [P, KO, QD], BF16, isOutput=False)
    bout_d = nc.declare_dram_parameter("bout", [P, QT], F32, isOutput=False)
    out0_d = nc.declare_dram_parameter("out0", [P, QT, FB], BF16, isOutput=True)
    out1_d = nc.declare_dram_parameter("out1", [P, QT, FB], BF16, isOutput=True)

    from contextlib import ExitStack

    with tile.TileContext(nc) as tc, ExitStack() as ctx:
        persist = ctx.enter_context(tc.tile_pool(name="persist", bufs=1))
        # PSUM budget (8 banks): sim 2x[128,1024]=4, attnv 2x[65,512]=2,
        # proj/out/normMM [128,512]x2=2
        pp_mm = ctx.enter_context(tc.tile_pool(name="pp_mm", bufs=2, space="PSUM"))
        pp_sim = ctx.enter_context(tc.tile_pool(name="pp_sim", bufs=2, space="PSUM"))
        pp_ob = ctx.enter_context(tc.tile_pool(name="pp_ob", bufs=2, space="PSUM"))
        sb_tmp = ctx.enter_context(tc.tile_pool(name="sb_tmp", bufs=2))
        expT_pool = ctx.enter_context(tc.tile_pool(name="expT", bufs=24))

        # ---- persist SBUF tiles; first-needed-first on each HWDGE queue ----
        Wq_sb = persist.tile([P, MI, KQ, P], BF16, tag="Wq", name="Wq")
        xT0_sb = persist.tile([P, KQ, FB], BF16, tag="xT0", name="xT0")
        Wk_sb = persist.tile([P, MI, KC, P], BF16, tag="Wk", name="Wk")
        ctxT_sb = persist.tile([P, KC, M], BF16, tag="ctxT", name="ctxT")
        Wv_sb = persist.tile([P, KC, INNER], BF16, tag="Wv", name="Wv")
        xT1_sb = persist.tile([P, KQ, FB], BF16, tag="xT1", name="xT1")
        Wout_sb = persist.tile([P, KO, QD], BF16, tag="Wout", name="Wout")
        bout_sb = persist.tile([P, QT], F32, tag="bout", name="bout")
        # scalar HWDGE ring
        nc.scalar.dma_start(out=Wq_sb[:, 0:1, :, :], in_=Wq0_d[:])
        nc.scalar.dma_start(out=xT0_sb[:, 0:KQH, :], in_=xT0a_d[:])
        nc.scalar.dma_start(out=Wk_sb[:, 0:1, :, :], in_=Wk0_d[:])
        nc.scalar.dma_start(out=Wk_sb[:, 1:MI, :, :], in_=Wkr_d[:])
        nc.scalar.dma_start(out=Wv_sb[:], in_=Wv_d[:])
        nc.scalar.dma_start(out=xT1_sb[:], in_=xT1_d[:])
        nc.scalar.dma_start(out=Wout_sb[:], in_=Wout_d[:])
        nc.scalar.dma_start(out=bout_sb[:], in_=bout_d[:])
        # sync HWDGE ring
        nc.sync.dma_start(out=xT0_sb[:, KQH:KQ, :], in_=xT0b_d[:])
        nc.sync.dma_start(out=ctxT_sb[:, :, 0:MH], in_=ctxTa_d[:])
        nc.sync.dma_start(out=ctxT_sb[:, :, MH:M], in_=ctxTb_d[:])
        nc.sync.dma_start(out=Wq_sb[:, 1:MI, :, :], in_=Wqr_d[:])
        xT_sb = [xT0_sb, xT1_sb]

        ones64 = persist.tile([1, 64], BF16, tag="ones64", name="ones64")
        nc.vector.memset(ones64[:], 1.0)

        # PE warm-up: HAM boots at 1.2 GHz; cover the initial DMA window.
        warm_w = persist.tile([P, FB], BF16, tag="warm", name="warm_w")
        nc.vector.memset(warm_w[:], 0.0)
        ps_w = pp_mm.tile([P, FB], F32, tag="mm", name="ps_w")
        for _ in range(4):
            nc.tensor.matmul(ps_w[:], warm_w[:, 0:P], warm_w[:], start=True, stop=True)
        warm_anchor = persist.tile([1, 1], F32, tag="warm_a", name="warm_anchor")
        nc.vector.tensor_copy(warm_anchor[:], ps_w[0:1, 0:1])

        vext_sb = []
        for i in range(MC):
            t = persist.tile([P, H, D + 1], BF16, tag=f"vext{i}", name=f"vext{i}")
            nc.vector.memset(t[:, :, D : D + 1], 1.0)
            vext_sb.append(t)

        qT_sb = [
            persist.tile([P, NSH], BF16, tag=f"qT{i}", name=f"qT{i}")
            for i in range(MI)
        ]
        kT_sb = [
            persist.tile([P, M], BF16, tag=f"kT{i}", name=f"kT{i}") for i in range(MI)
        ]
        oT_sb = [
            persist.tile([P, NSH], BF16, tag=f"oT{i}", name=f"oT{i}")
            for i in range(MI)
        ]
        stage_sb = [
            persist.tile([P, QT, FB], BF16, tag=f"stage{nb}", name=f"stage{nb}")
            for nb in range(NB)
        ]

        exp_tiles = {}  # (pair, nb) -> [MC tiles]
        ob_tiles = {}  # (pair, nb) -> (ps_a, ps_b)

        # ---- projection chains as thunk lists (mm*K + evacuation copy) ----
        def chain_q(mi, nb):
            st = {}

            def mk(k):
                def f():
                    if k == 0:
                        st["ps"] = pp_mm.tile(
                            [P, FB], F32, tag="mm", name=f"ps_q{mi}_{nb}"
                        )
                    nc.tensor.matmul(
                        st["ps"][:],
                        Wq_sb[:, mi, k, :],
                        xT_sb[nb][:, k, :],
                        start=(k == 0),
                        stop=(k == KQ - 1),
                    )

                return f

            def cp():
                nc.vector.tensor_copy(
                    qT_sb[mi][:, nb * FB : (nb + 1) * FB], st["ps"][:]
                )

            return [mk(k) for k in range(KQ)] + [cp]

        def chain_k(mi, mb):
            st = {}

            def mk(k):
                def f():
                    if k == 0:
                        st["ps"] = pp_mm.tile(
                            [P, FB], F32, tag="mm", name=f"ps_k{mi}_{mb}"
                        )
                    nc.tensor.matmul(
                        st["ps"][:],
                        Wk_sb[:, mi, k, :],
                        ctxT_sb[:, k, mb * FB : (mb + 1) * FB],
                        start=(k == 0),
                        stop=(k == KC - 1),
                    )

                return f

            def cp():
                nc.vector.tensor_copy(
                    kT_sb[mi][:, mb * FB : (mb + 1) * FB], st["ps"][:]
                )

            return [mk(k) for k in range(KC)] + [cp]

        def chain_v(t_i):
            st = {}

            def mk(k):
                def f():
                    if k == 0:
                        st["ps"] = pp_mm.tile([P, FB], F32, tag="mm", name=f"ps_v{t_i}")
                    nc.tensor.matmul(
                        st["ps"][:],
                        ctxT_sb[:, k, t_i * P : (t_i + 1) * P],
                        Wv_sb[:, k, :],
                        start=(k == 0),
                        stop=(k == KC - 1),
                    )

                return f

            def cp():
                nc.vector.tensor_copy(
                    vext_sb[t_i][:, :, 0:D],
                    st["ps"][:].rearrange("p (h d) -> p h d", h=H),
                )

            return [mk(k) for k in range(KC)] + [cp]

        # ---- attention emitters ----
        def sim_one(pair, nb, mc):
            ps = pp_sim.tile([P, NSH], F32, tag="sim", name=f"ps_s{pair}_{nb}_{mc}")
            for j in range(2):
                nc.tensor.matmul(
                    ps[:, j * FB : (j + 1) * FB],
                    kT_sb[pair][j * D : (j + 1) * D, mc * P : (mc + 1) * P],
                    qT_sb[pair][j * D : (j + 1) * D, nb * FB : (nb + 1) * FB],
                    start=True,
                    stop=True,
                )
            et = expT_pool.tile(
                [P, NSH], BF16, tag="expT", name=f"exp{pair}_{nb}_{mc}"
            )
            nc.scalar.activation(
                et[:], ps[:], mybir.ActivationFunctionType.Exp, scale=float(D) ** -0.5
            )
            exp_tiles[(pair, nb)][mc] = et

        def alloc_ob(pair, nb):
            pa = pp_ob.tile([D + 1, FB], F32, tag="ob", name=f"ps_a{pair}_{nb}")
            pb = pp_ob.tile([D + 1, FB], F32, tag="ob", name=f"ps_b{pair}_{nb}")
            ob_tiles[(pair, nb)] = (pa, pb)
            exp_tiles.setdefault((pair, nb), [None] * MC)

        def av(pair, nb, mc):
            # both heads' attnv for one m-chunk (2 thunks)
            def mk(j):
                def f():
                    ps_o = ob_tiles[(pair, nb)][j]
                    h = 2 * pair + j
                    nc.tensor.matmul(
                        ps_o[:],
                        vext_sb[mc][:, h : h + 1, :],
                        exp_tiles[(pair, nb)][mc][:, j * FB : (j + 1) * FB],
                        start=(mc == 0),
                        stop=(mc == MC - 1),
                    )

                return f

            return [mk(0), mk(1)]

        # ---- batched normalize: denominators -> reciprocal -> broadcast ----
        def norm_steps(pair, nb):
            st = {}

            def s1():
                # bounce denominators through SBUF (custom-DVE reciprocal
                # reads garbage from a PSUM source — HW-verified)
                pa, pb = ob_tiles[(pair, nb)]
                S = sb_tmp.tile([1, 2, FB], F32, tag="S", name=f"S{pair}_{nb}")
                nc.vector.tensor_copy(S[:, 0, :], pa[D : D + 1, :])
                nc.vector.tensor_copy(S[:, 1, :], pb[D : D + 1, :])
                R = sb_tmp.tile([1, 2, FB], F32, tag="R", name=f"R{pair}_{nb}")
                nc.vector.reciprocal_approx_fast(out=R[:], in_=S[:])
                Rb = sb_tmp.tile([1, 2, FB], BF16, tag="Rb", name=f"Rb{pair}_{nb}")
                nc.vector.tensor_copy(Rb[:], R[:])
                st["Rb"] = Rb

            def s2():
                # two K=1 broadcast matmuls into disjoint col-groups of one
                # psum tile — they run concurrently in the PE array
                ps_rb = pp_mm.tile([P, FB], F32, tag="mm", name=f"ps_rb{pair}_{nb}")
                nc.tensor.matmul(
                    ps_rb[0:D, :], ones64[:], st["Rb"][:, 0, :], start=True, stop=True
                )
                nc.tensor.matmul(
                    ps_rb[D:P, :], ones64[:], st["Rb"][:, 1, :], start=True, stop=True
                )
                st["ps_rb"] = ps_rb

            def s3a():
                rB = sb_tmp.tile([P, FB], F32, tag="recipB", name=f"rB{pair}_{nb}")
                nc.vector.tensor_copy(rB[:], st["ps_rb"][:])
                st["rB"] = rB

            def s3b():
                pa, pb = ob_tiles[(pair, nb)]
                rB = st["rB"]
                nc.vector.tensor_mul(
                    oT_sb[pair][0:D, nb * FB : (nb + 1) * FB], pa[0:D, :], rB[0:D, :]
                )
                nc.vector.tensor_mul(
                    oT_sb[pair][D:P, nb * FB : (nb + 1) * FB], pb[0:D, :], rB[D:P, :]
                )

            return s1, s2, s3a, s3b

        def dummy_mms(n):
            def f():
                ps_d = pp_mm.tile([P, FB], F32, tag="mm", name="ps_dummy")
                for _ in range(n):
                    nc.tensor.matmul(
                        ps_d[:], warm_w[:, 0:P], warm_w[:], start=True, stop=True
                    )

            return f

        # ---- out-projection chain (4 mm + bias-add on ScalarE + dma) ----
        def chain_o(mi, nb, dma_eng):
            st = {}

            def mk(k):
                def f():
                    if k == 0:
                        st["ps"] = pp_mm.tile(
                            [P, FB], F32, tag="mm", name=f"ps_o{mi}_{nb}"
                        )
                    nc.tensor.matmul(
                        st["ps"][:],
                        Wout_sb[:, k, mi * P : (mi + 1) * P],
                        oT_sb[k][:, nb * FB : (nb + 1) * FB],
                        start=(k == 0),
                        stop=(k == KO - 1),
                    )

                return f

            def fin():
                nc.scalar.add(
                    stage_sb[nb][:, mi, :], st["ps"][:], bout_sb[:, mi : mi + 1]
                )
                eng = nc.sync if dma_eng == "s" else nc.scalar
                eng.dma_start(
                    out=(out0_d if nb == 0 else out1_d)[:, mi, :],
                    in_=stage_sb[nb][:, mi, :],
                )

            return [mk(k) for k in range(KO)] + [fin]

        # ---- phase driver ----
        def run_phase(pair, nb, slots, post=(), pre=()):
            exp_tiles.setdefault((pair, nb), [None] * MC)
            for f in pre:
                f()
            for mc in range(MC):
                sim_one(pair, nb, mc)
                for f in slots[mc]:
                    f()
            for f in post:
                f()

        # ================= prologue =================
        for f in chain_q(0, 0):
            f()
        for f in chain_k(0, 0):
            f()

        # ================= pair 0, phase A (nb0 sims) =================
        alloc_ob(0, 0)
        kc10 = chain_k(1, 0)
        kc01 = chain_k(0, 1)
        qc01 = chain_q(0, 1)
        qc10 = chain_q(1, 0)
        vch = [chain_v(i) for i in range(MC)]
        run_phase(
            0,
            0,
            [
                [],
                [],
                [],
                kc01[0:7],
                [],
                vch[0][0:7],
                kc10[0:7],
                qc01[0:4],
            ],
            post=qc01[4:9] + qc10[0:9] + vch[1][0:7] + vch[2][0:7] + vch[3][0:7],
        )

        # ================= pair 0, phase B (nb1 sims) =================
        n1, n2, n3a, n3b = norm_steps(0, 0)
        run_phase(
            0,
            1,
            [
                av(0, 0, 0) + vch[4][0:5],
                av(0, 0, 1) + vch[4][5:7] + vch[5][0:3],
                av(0, 0, 2) + vch[5][3:7] + vch[6][0:2],
                av(0, 0, 3) + av(0, 0, 4) + vch[6][2:7],
                av(0, 0, 5) + vch[7][0:5],
                av(0, 0, 6) + vch[7][5:7],
                av(0, 0, 7) + [n1],
                [n2, n3a],
            ],
            post=[n3b],
        )

        # ================= pairs 1..3 =================
        for p in range(1, MI):
            # ---- phase A: sims nb0; prev pair's nb1 attnv + norms ----
            alloc_ob(p - 1, 1)
            na1, na2, na3a, na3b = norm_steps(p - 1, 1)
            kcb = chain_k(p, 1)
            qcb = chain_q(p, 1)
            run_phase(
                p,
                0,
                [
                    av(p - 1, 1, 0) + av(p - 1, 1, 1),
                    av(p - 1, 1, 2) + av(p - 1, 1, 3),
                    av(p - 1, 1, 4) + av(p - 1, 1, 5),
                    av(p - 1, 1, 6) + av(p - 1, 1, 7) + qcb[0:2],
                    [na1] + qcb[2:5],
                    [na2, na3a] + qcb[5:7],
                    [na3b] + qcb[7:9],
                    [],
                ],
                pre=kcb[0:7],
            )
            # ---- phase B: sims nb1; this pair's nb0 attnv ----
            alloc_ob(p, 0)
            nb1, nb2, nb3a, nb3b = norm_steps(p, 0)
            if p < MI - 1:
                kcn = chain_k(p + 1, 0)
                qcn = chain_q(p + 1, 0)
                slots = [
                    av(p, 0, 0) + av(p, 0, 1),
                    av(p, 0, 2) + av(p, 0, 3),
                    av(p, 0, 4) + av(p, 0, 5),
                    av(p, 0, 6) + av(p, 0, 7) + qcn[0:2],
                    [nb1] + qcn[2:5],
                    [nb2, nb3a] + qcn[5:7],
                    [nb3b] + qcn[7:9],
                    [],
                ]
                run_phase(p, 1, slots, pre=kcn[0:7])
            else:
                slots = [
                    av(p, 0, 0) + av(p, 0, 1),
                    av(p, 0, 2) + av(p, 0, 3),
                    av(p, 0, 4) + av(p, 0, 5),
                    av(p, 0, 6) + av(p, 0, 7),
                    [nb1, dummy_mms(3)],
                    [nb2, nb3a, dummy_mms(3)],
                    [nb3b, dummy_mms(3)],
                    [dummy_mms(3)],
                ]
                run_phase(p, 1, slots)

        # ================= epilogue =================
        # last pair's nb1 attnv interleaved with the nb0 out-proj wave
        alloc_ob(MI - 1, 1)
        ne1, ne2, ne3a, ne3b = norm_steps(MI - 1, 1)
        for mc in range(MC):
            for f in av(MI - 1, 1, mc):
                f()
            if mc == MC - 1:
                ne1()
            for f in chain_o(mc, 0, "s" if mc % 2 == 0 else "a"):
                f()
        ne2()
        ne3a()
        ne3b()
        for mi in range(QT):
            for f in chain_o(mi, 1, "s" if mi % 2 == 0 else "a"):
                f()

    nc.compile()
    return nc


_NC_CACHE = None


def _get_nc():
    global _NC_CACHE
    if _NC_CACHE is None:
        _NC_CACHE = build_nc()
    return _NC_CACHE


def make_in_maps(x, context, Wq, Wk, Wv, Wout, bout):
    bf = ml_dtypes.bfloat16
    # pair-major weight layouts: [P, MI, K, P]
    Wq_b = np.ascontiguousarray(
        Wq.reshape(KQ, P, MI, P).transpose(1, 2, 0, 3)
    ).astype(bf)
    Wk_b = np.ascontiguousarray(
        Wk.reshape(KC, P, MI, P).transpose(1, 2, 0, 3)
    ).astype(bf)
    Wv_b = np.ascontiguousarray(Wv.reshape(KC, P, INNER).transpose(1, 0, 2)).astype(bf)
    Wout_b = np.ascontiguousarray(
        Wout.reshape(KO, P, QD).transpose(1, 0, 2)
    ).astype(bf)
    bout_r = np.ascontiguousarray(bout.reshape(QT, P).T, dtype=np.float32)
    in_maps = []
    for c in range(8):
        b, half = divmod(c, 2)
        xh = x[b, half * NSH : (half + 1) * NSH, :]  # [NSH, QD]
        xr = xh.reshape(NB, FB, KQ, P).transpose(3, 2, 0, 1)  # [P, KQ, NB, FB]
        xT0 = np.ascontiguousarray(xr[:, :, 0, :]).astype(bf)
        xT1 = np.ascontiguousarray(xr[:, :, 1, :]).astype(bf)
        ctxT = np.ascontiguousarray(
            context[b].reshape(M, KC, P).transpose(2, 1, 0)
        ).astype(bf)
        in_maps.append(
            {
                "Wq0": np.ascontiguousarray(Wq_b[:, 0:1, :, :]),
                "Wqr": np.ascontiguousarray(Wq_b[:, 1:MI, :, :]),
                "xT0a": np.ascontiguousarray(xT0[:, 0:KQH, :]),
                "xT0b": np.ascontiguousarray(xT0[:, KQH:KQ, :]),
                "Wk0": np.ascontiguousarray(Wk_b[:, 0:1, :, :]),
                "Wkr": np.ascontiguousarray(Wk_b[:, 1:MI, :, :]),
                "ctxTa": np.ascontiguousarray(ctxT[:, :, 0:MH]),
                "ctxTb": np.ascontiguousarray(ctxT[:, :, MH:M]),
                "Wv": Wv_b,
                "xT1": xT1,
                "Wout": Wout_b,
                "bout": bout_r,
            }
        )
    return in_maps


def gather_out(results):
    out = np.empty((B, N, QD), dtype=np.float32)
    for c in range(8):
        b, half = divmod(c, 2)
        for nb, key in ((0, "out0"), (1, "out1")):
            blk = results[c][key].astype(np.float32)  # [P, QT, FB]
            out[b, half * NSH + nb * FB : half * NSH + (nb + 1) * FB, :] = (
                blk.transpose(2, 1, 0).reshape(FB, QD)
            )
    return out


def kernel(**inputs):
    nc = _get_nc()
    in_maps = make_in_maps(**inputs)
    res = run_bass_kernel_spmd(nc, in_maps, list(range(8)))
    return gather_out(res.results)


if __name__ == "__main__":
    rng = np.random.default_rng(0)
    ins = {
        "x": rng.standard_normal((B, N, QD), dtype=np.float32),
        "context": rng.standard_normal((B, M, CD), dtype=np.float32),
        "Wq": rng.standard_normal((QD, INNER), dtype=np.float32) / 32,
        "Wk": rng.standard_normal((CD, INNER), dtype=np.float32) / 27.7,
        "Wv": rng.standard_normal((CD, INNER), dtype=np.float32) / 27.7,
        "Wout": rng.standard_normal((INNER, QD), dtype=np.float32) / 22.6,
        "bout": rng.standard_normal((QD,), dtype=np.float32) * 0.01,
    }
    out = kernel(**ins)
    print("out", out.shape, out.dtype, np.abs(out).mean())


# revision 17
# speedup vs baseline: 1.0640x; 1.0640x over previous
"""CrossAttention Trainium2 kernel, SPMD over 8 NeuronCores.

Problem: x[4,2048,1024], context[4,1024,768], Wq[1024,512], Wk/Wv[768,512],
Wout[512,1024], bout[1024] -> out[4,2048,1024] (f32).

Sharding: 8 cores = 4 batches x 2 halves of the query dim n (2048 -> 2x1024).
Each core computes full attention for its (batch, n-half) with no collectives.

v6 design notes (trace-driven, evolved from v5):
- Startup is device-HBM-bound (8 cores x 7MB saturate HBM ~20us). Wq/Wk
  are stored pair-major so the 448KB the first sims need lands first;
  first exp fires ~14us instead of ~28us.
- ScalarE exp paces each phase (8 x ~1.08us). attnv for the previously
  simmed block + projection chains + the batched normalize chain are
  spread across slots so the PE never drains at phase boundaries (HAM
  stays at K=8/8).
- Batched normalize: both denominators -> SBUF bounce (custom-DVE recip
  reads garbage from PSUM), one [1,2,FB] reciprocal, one bf16 cast, two
  concurrent K=1 col-group broadcast MMs, one evacuation, two muls.
- Every attnv+normalize completes inside its phase, so the pp_ob psum
  rotation never blocks the next phase.
- Epilogue: outproj chains interleave with the last attnv block; all
  bias-adds ride the otherwise-idle ScalarE; dummy warm MMs keep HAM
  hot through pair-3 phase B.
"""

import numpy as np
import ml_dtypes

import concourse.bass as bass
import concourse.mybir as mybir
import concourse.tile as tile
from concourse import bacc
from concourse.bass_utils import run_bass_kernel_spmd

BF16 = mybir.dt.bfloat16
F32 = mybir.dt.float32

B, N, QD = 4, 2048, 1024
M, CD = 1024, 768
H, D = 8, 64
INNER = H * D  # 512
NSH = N // 2  # 1024 query rows per core
P = 128
FB = 512  # psum bank = 512 f32

KQ = QD // P  # 8 contraction tiles for q-proj
KC = CD // P  # 6 contraction tiles for k/v-proj
MI = INNER // P  # 4 head pairs
NB = NSH // FB  # 2 n blocks
MC = M // P  # 8 m chunks
KO = INNER // P  # 4 contraction tiles for out-proj
QT = QD // P  # 8 out-proj row tiles
MH = M // 2  # 512, ctxT descriptor split
KQH = KQ // 2


def build_nc():
    nc = bacc.Bacc(None)

    Wq0_d = nc.declare_dram_parameter("Wq0", [P, 1, KQ, P], BF16, isOutput=False)
    Wqr_d = nc.declare_dram_parameter("Wqr", [P, MI - 1, KQ, P], BF16, isOutput=False)
    xT0a_d = nc.declare_dram_parameter("xT0a", [P, KQH, FB], BF16, isOutput=False)
    xT0b_d = nc.declare_dram_parameter("xT0b", [P, KQH, FB], BF16, isOutput=False)
    Wk0_d = nc.declare_dram_parameter("Wk0", [P, 1, KC, P], BF16, isOutput=False)
    Wkr_d = nc.declare_dram_parameter("Wkr", [P, MI - 1, KC, P], BF16, isOutput=False)
    ctxTa_d = nc.declare_dram_parameter("ctxTa", [P, KC, MH], BF16, isOutput=False)
    ctxTb_d = nc.declare_dram_parameter("ctxTb", [P, KC, MH], BF16, isOutput=False)
    Wv_d = nc.declare_dram_parameter("Wv", [P, KC, INNER], BF16, isOutput=False)
    xT1_d = nc.declare_dram_parameter("xT1", [P, KQ, FB], BF16, isOutput=False)
    Wout_d = nc.declare_dram_parameter("Wout", [P, KO, QD], BF16, isOutput=False)
    bout_d = nc.declare_dram_parameter("bout", [P, QT], F32, isOutput=False)
    out0_d = nc.declare_dram_parameter("out0", [P, QT, FB], BF16, isOutput=True)
    out1_d = nc.declare_dram_parameter("out1", [P, QT, FB], BF16, isOutput=True)

    from contextlib import ExitStack

    with tile.TileContext(nc) as tc, ExitStack() as ctx:
        persist = ctx.enter_context(tc.tile_pool(name="persist", bufs=1))
        # PSUM budget (8 banks): sim 2x[128,1024]=4, attnv 2x[65,512]=2,
        # proj/out/normMM [128,512]x2=2
        pp_mm = ctx.enter_context(tc.tile_pool(name="pp_mm", bufs=2, space="PSUM"))
        pp_sim = ctx.enter_context(tc.tile_pool(name="pp_sim", bufs=2, space="PSUM"))
        pp_ob = ctx.enter_context(tc.tile_pool(name="pp_ob", bufs=2, space="PSUM"))
        sb_tmp = ctx.enter_context(tc.tile_pool(name="sb_tmp", bufs=2))
        expT_pool = ctx.enter_context(tc.tile_pool(name="expT", bufs=24))

        # ---- persist SBUF tiles; first-needed-first on each HWDGE queue ----
        Wq_sb = persist.tile([P, MI, KQ, P], BF16, tag="Wq", name="Wq")
        xT0_sb = persist.tile([P, KQ, FB], BF16, tag="xT0", name="xT0")
        Wk_sb = persist.tile([P, MI, KC, P], BF16, tag="Wk", name="Wk")
        ctxT_sb = persist.tile([P, KC, M], BF16, tag="ctxT", name="ctxT")
        Wv_sb = persist.tile([P, KC, INNER], BF16, tag="Wv", name="Wv")
        xT1_sb = persist.tile([P, KQ, FB], BF16, tag="xT1", name="xT1")
        Wout_sb = persist.tile([P, KO, QD], BF16, tag="Wout", name="Wout")
        bout_sb = persist.tile([P, QT], F32, tag="bout", name="bout")
        # scalar HWDGE ring
        nc.scalar.dma_start(out=Wq_sb[:, 0:1, :, :], in_=Wq0_d[:])
        nc.scalar.dma_start(out=xT0_sb[:, 0:KQH, :], in_=xT0a_d[:])
        nc.scalar.dma_start(out=Wk_sb[:, 0:1, :, :], in_=Wk0_d[:])
        nc.scalar.dma_start(out=Wk_sb[:, 1:MI, :, :], in_=Wkr_d[:])
        nc.scalar.dma_start(out=Wv_sb[:], in_=Wv_d[:])
        nc.scalar.dma_start(out=xT1_sb[:], in_=xT1_d[:])
        nc.scalar.dma_start(out=Wout_sb[:], in_=Wout_d[:])
        nc.scalar.dma_start(out=bout_sb[:], in_=bout_d[:])
        # sync HWDGE ring
        nc.sync.dma_start(out=xT0_sb[:, KQH:KQ, :], in_=xT0b_d[:])
        nc.sync.dma_start(out=ctxT_sb[:, :, 0:MH], in_=ctxTa_d[:])
        nc.sync.dma_start(out=ctxT_sb[:, :, MH:M], in_=ctxTb_d[:])
        nc.sync.dma_start(out=Wq_sb[:, 1:MI, :, :], in_=Wqr_d[:])
        xT_sb = [xT0_sb, xT1_sb]

        ones64 = persist.tile([1, 64], BF16, tag="ones64", name="ones64")
        nc.vector.memset(ones64[:], 1.0)

        # PE warm-up: HAM boots at 1.2 GHz; cover the initial DMA window.
        warm_w = persist.tile([P, FB], BF16, tag="warm", name="warm_w")
        nc.vector.memset(warm_w[:], 0.0)
        ps_w = pp_mm.tile([P, FB], F32, tag="mm", name="ps_w")
        for _ in range(4):
            nc.tensor.matmul(ps_w[:], warm_w[:, 0:P], warm_w[:], start=True, stop=True)
        warm_anchor = persist.tile([1, 1], F32, tag="warm_a", name="warm_anchor")
        nc.vector.tensor_copy(warm_anchor[:], ps_w[0:1, 0:1])

        vext_sb = []
        for i in range(MC):
            t = persist.tile([P, H, D + 1], BF16, tag=f"vext{i}", name=f"vext{i}")
            nc.vector.memset(t[:, :, D : D + 1], 1.0)
            vext_sb.append(t)

        qT_sb = [
            persist.tile([P, NSH], BF16, tag=f"qT{i}", name=f"qT{i}")
            for i in range(MI)
        ]
        kT_sb = [
            persist.tile([P, M], BF16, tag=f"kT{i}", name=f"kT{i}") for i in range(MI)
        ]
        oT_sb = [
            persist.tile([P, NSH], BF16, tag=f"oT{i}", name=f"oT{i}")
            for i in range(MI)
        ]
        stage_sb = [
            persist.tile([P, QT, FB], BF16, tag=f"stage{nb}", name=f"stage{nb}")
            for nb in range(NB)
        ]

        exp_tiles = {}  # (pair, nb) -> [MC tiles]
        ob_tiles = {}  # (pair, nb) -> (ps_a, ps_b)

        # ---- projection chains as thunk lists (mm*K + evacuation copy) ----
        def chain_q(mi, nb):
            st = {}

            def mk(k):
                def f():
                    if k == 0:
                        st["ps"] = pp_mm.tile(
                            [P, FB], F32, tag="mm", name=f"ps_q{mi}_{nb}"
                        )
                    nc.tensor.matmul(
                        st["ps"][:],
                        Wq_sb[:, mi, k, :],
                        xT_sb[nb][:, k, :],
                        start=(k == 0),
                        stop=(k == KQ - 1),
                    )

                return f

            def cp():
                nc.vector.tensor_copy(
                    qT_sb[mi][:, nb * FB : (nb + 1) * FB], st["ps"][:]
                )

            return [mk(k) for k in range(KQ)] + [cp]

        def chain_k(mi, mb):
            st = {}

            def mk(k):
                def f():
                    if k == 0:
                        st["ps"] = pp_mm.tile(
                            [P, FB], F32, tag="mm", name=f"ps_k{mi}_{mb}"
                        )
                    nc.tensor.matmul(
                        st["ps"][:],
                        Wk_sb[:, mi, k, :],
                        ctxT_sb[:, k, mb * FB : (mb + 1) * FB],
                        start=(k == 0),
                        stop=(k == KC - 1),
                    )

                return f

            def cp():
                nc.vector.tensor_copy(
                    kT_sb[mi][:, mb * FB : (mb + 1) * FB], st["ps"][:]
                )

            return [mk(k) for k in range(KC)] + [cp]

        def chain_v(t_i):
            st = {}

            def mk(k):
                def f():
                    if k == 0:
                        st["ps"] = pp_mm.tile([P, FB], F32, tag="mm", name=f"ps_v{t_i}")
                    nc.tensor.matmul(
                        st["ps"][:],
                        ctxT_sb[:, k, t_i * P : (t_i + 1) * P],
                        Wv_sb[:, k, :],
                        start=(k == 0),
                        stop=(k == KC - 1),
                    )

                return f

            def cp():
                nc.vector.tensor_copy(
                    vext_sb[t_i][:, :, 0:D],
                    st["ps"][:].rearrange("p (h d) -> p h d", h=H),
                )

            return [mk(k) for k in range(KC)] + [cp]

        # ---- attention emitters ----
        def sim_one(pair, nb, mc):
            ps = pp_sim.tile([P, NSH], F32, tag="sim", name=f"ps_s{pair}_{nb}_{mc}")
            for j in range(2):
                nc.tensor.matmul(
                    ps[:, j * FB : (j + 1) * FB],
                    kT_sb[pair][j * D : (j + 1) * D, mc * P : (mc + 1) * P],
                    qT_sb[pair][j * D : (j + 1) * D, nb * FB : (nb + 1) * FB],
                    start=True,
                    stop=True,
                )
            et = expT_pool.tile(
                [P, NSH], BF16, tag="expT", name=f"exp{pair}_{nb}_{mc}"
            )
            nc.scalar.activation(
                et[:], ps[:], mybir.ActivationFunctionType.Exp, scale=float(D) ** -0.5
            )
            exp_tiles[(pair, nb)][mc] = et

        def alloc_ob(pair, nb):
            pa = pp_ob.tile([D + 1, FB], F32, tag="ob", name=f"ps_a{pair}_{nb}")
            pb = pp_ob.tile([D + 1, FB], F32, tag="ob", name=f"ps_b{pair}_{nb}")
            ob_tiles[(pair, nb)] = (pa, pb)
            exp_tiles.setdefault((pair, nb), [None] * MC)

        def av(pair, nb, mc):
            # both heads' attnv for one m-chunk (2 thunks)
            def mk(j):
                def f():
                    ps_o = ob_tiles[(pair, nb)][j]
                    h = 2 * pair + j
                    nc.tensor.matmul(
                        ps_o[:],
                        vext_sb[mc][:, h : h + 1, :],
                        exp_tiles[(pair, nb)][mc][:, j * FB : (j + 1) * FB],
                        start=(mc == 0),
                        stop=(mc == MC - 1),
                    )

                return f

            return [mk(0), mk(1)]

        # ---- batched normalize: denominators -> reciprocal -> broadcast ----
        def norm_steps(pair, nb):
            st = {}

            def s1():
                # bounce denominators through SBUF (custom-DVE reciprocal
                # reads garbage from a PSUM source — HW-verified)
                pa, pb = ob_tiles[(pair, nb)]
                S = sb_tmp.tile([1, 2, FB], F32, tag="S", name=f"S{pair}_{nb}")
                nc.vector.tensor_copy(S[:, 0, :], pa[D : D + 1, :])
                nc.vector.tensor_copy(S[:, 1, :], pb[D : D + 1, :])
                R = sb_tmp.tile([1, 2, FB], F32, tag="R", name=f"R{pair}_{nb}")
                nc.vector.reciprocal_approx_fast(out=R[:], in_=S[:])
                Rb = sb_tmp.tile([1, 2, FB], BF16, tag="Rb", name=f"Rb{pair}_{nb}")
                nc.vector.tensor_copy(Rb[:], R[:])
                st["Rb"] = Rb

            def s2():
                # two K=1 broadcast matmuls into disjoint col-groups of one
                # psum tile — they run concurrently in the PE array
                ps_rb = pp_mm.tile([P, FB], F32, tag="mm", name=f"ps_rb{pair}_{nb}")
                nc.tensor.matmul(
                    ps_rb[0:D, :], ones64[:], st["Rb"][:, 0, :], start=True, stop=True
                )
                nc.tensor.matmul(
                    ps_rb[D:P, :], ones64[:], st["Rb"][:, 1, :], start=True, stop=True
                )
                st["ps_rb"] = ps_rb

            def s3a():
                rB = sb_tmp.tile([P, FB], F32, tag="recipB", name=f"rB{pair}_{nb}")
                nc.vector.tensor_copy(rB[:], st["ps_rb"][:])
                st["rB"] = rB

            def s3b():
                pa, pb = ob_tiles[(pair, nb)]
                rB = st["rB"]
                nc.vector.tensor_mul(
                    oT_sb[pair][0:D, nb * FB : (nb + 1) * FB], pa[0:D, :], rB[0:D, :]
                )
                nc.vector.tensor_mul(
                    oT_sb[pair][D:P, nb * FB : (nb + 1) * FB], pb[0:D, :], rB[D:P, :]
                )

            return s1, s2, s3a, s3b

        def dummy_mms(n):
            def f():
                ps_d = pp_mm.tile([P, FB], F32, tag="mm", name="ps_dummy")
                for _ in range(n):
                    nc.tensor.matmul(
                        ps_d[:], warm_w[:, 0:P], warm_w[:], start=True, stop=True
                    )

            return f

        # ---- out-projection chain (4 mm + bias-add on ScalarE + dma) ----
        def chain_o(mi, nb, dma_eng):
            st = {}

            def mk(k):
                def f():
                    if k == 0:
                        st["ps"] = pp_mm.tile(
                            [P, FB], F32, tag="mm", name=f"ps_o{mi}_{nb}"
                        )
                    nc.tensor.matmul(
                        st["ps"][:],
                        Wout_sb[:, k, mi * P : (mi + 1) * P],
                        oT_sb[k][:, nb * FB : (nb + 1) * FB],
                        start=(k == 0),
                        stop=(k == KO - 1),
                    )

                return f

            def fin():
                nc.scalar.add(
                    stage_sb[nb][:, mi, :], st["ps"][:], bout_sb[:, mi : mi + 1]
                )
                eng = nc.sync if dma_eng == "s" else nc.scalar
                eng.dma_start(
                    out=(out0_d if nb == 0 else out1_d)[:, mi, :],
                    in_=stage_sb[nb][:, mi, :],
                )

            return [mk(k) for k in range(KO)] + [fin]

        # ---- phase driver ----
        def run_phase(pair, nb, slots, post=(), pre=()):
            exp_tiles.setdefault((pair, nb), [None] * MC)
            for f in pre:
                f()
            for mc in range(MC):
                sim_one(pair, nb, mc)
                for f in slots[mc]:
                    f()
            for f in post:
                f()

        # ================= prologue =================
        for f in chain_q(0, 0):
            f()
        for f in chain_k(0, 0):
            f()

        # ================= pair 0, phase A (nb0 sims) =================
        alloc_ob(0, 0)
        kc10 = chain_k(1, 0)
        kc01 = chain_k(0, 1)
        qc01 = chain_q(0, 1)
        qc10 = chain_q(1, 0)
        vch = [chain_v(i) for i in range(MC)]
        run_phase(
            0,
            0,
            [
                kc10[0:4],
                kc10[4:7] + kc01[0:2],
                kc01[2:7],
                vch[0][0:4],
                vch[0][4:7] + vch[1][0:2],
                vch[1][2:7],
                vch[2][0:5],
                vch[2][5:7] + qc01[0:3],
            ],
            post=qc01[3:9] + qc10[0:9] + vch[3][0:7],
        )

        # ================= pair 0, phase B (nb1 sims) =================
        n1, n2, n3a, n3b = norm_steps(0, 0)
        run_phase(
            0,
            1,
            [
                av(0, 0, 0) + vch[4][0:5],
                av(0, 0, 1) + vch[4][5:7] + vch[5][0:3],
                av(0, 0, 2) + vch[5][3:7] + vch[6][0:2],
                av(0, 0, 3) + av(0, 0, 4) + vch[6][2:7],
                av(0, 0, 5) + vch[7][0:5],
                av(0, 0, 6) + vch[7][5:7],
                av(0, 0, 7) + [n1],
                [n2, n3a],
            ],
            post=[n3b],
        )

        # ================= pairs 1..3 =================
        for p in range(1, MI):
            # ---- phase A: sims nb0; prev pair's nb1 attnv + norms ----
            alloc_ob(p - 1, 1)
            na1, na2, na3a, na3b = norm_steps(p - 1, 1)
            kcb = chain_k(p, 1)
            qcb = chain_q(p, 1)
            run_phase(
                p,
                0,
                [
                    av(p - 1, 1, 0) + av(p - 1, 1, 1),
                    av(p - 1, 1, 2) + av(p - 1, 1, 3),
                    av(p - 1, 1, 4) + av(p - 1, 1, 5),
                    av(p - 1, 1, 6) + av(p - 1, 1, 7) + qcb[0:2],
                    [na1] + qcb[2:5],
                    [na2, na3a] + qcb[5:7],
                    [na3b] + qcb[7:9],
                    [],
                ],
                pre=kcb[0:7],
            )
            # ---- phase B: sims nb1; this pair's nb0 attnv ----
            alloc_ob(p, 0)
            nb1, nb2, nb3a, nb3b = norm_steps(p, 0)
            if p < MI - 1:
                kcn = chain_k(p + 1, 0)
                qcn = chain_q(p + 1, 0)
                slots = [
                    av(p, 0, 0) + av(p, 0, 1),
                    av(p, 0, 2) + av(p, 0, 3),
                    av(p, 0, 4) + av(p, 0, 5),
                    av(p, 0, 6) + av(p, 0, 7) + qcn[0:2],
                    [nb1] + qcn[2:5],
                    [nb2, nb3a] + qcn[5:7],
                    [nb3b] + qcn[7:9],
                    [],
                ]
                run_phase(p, 1, slots, pre=kcn[0:7])
            else:
                slots = [
                    av(p, 0, 0) + av(p, 0, 1),
                    av(p, 0, 2) + av(p, 0, 3),
                    av(p, 0, 4) + av(p, 0, 5),
                    av(p, 0, 6) + av(p, 0, 7),
                    [nb1, dummy_mms(3)],
                    [nb2, nb3a, dummy_mms(3)],
                    [nb3b, dummy_mms(3)],
                    [dummy_mms(3)],
                ]
                run_phase(p, 1, slots)

        # ================= epilogue =================
        # last pair's nb1 attnv interleaved with the nb0 out-proj wave
        alloc_ob(MI - 1, 1)
        ne1, ne2, ne3a, ne3b = norm_steps(MI - 1, 1)
        for mc in range(MC):
            for f in av(MI - 1, 1, mc):
                f()
            if mc == MC - 1:
                ne1()
            for f in chain_o(mc, 0, "s" if mc % 2 == 0 else "a"):
                f()
        ne2()
        ne3a()
        ne3b()
        for mi in range(QT):
            for f in chain_o(mi, 1, "s" if mi % 2 == 0 else "a"):
                f()

    nc.compile()
    return nc


_NC_CACHE = None


def _get_nc():
    global _NC_CACHE
    if _NC_CACHE is None:
        _NC_CACHE = build_nc()
    return _NC_CACHE


def make_in_maps(x, context, Wq, Wk, Wv, Wout, bout):
    bf = ml_dtypes.bfloat16
    # pair-major weight layouts: [P, MI, K, P]
    Wq_b = np.ascontiguousarray(
        Wq.reshape(KQ, P, MI, P).transpose(1, 2, 0, 3)
    ).astype(bf)
    Wk_b = np.ascontiguousarray(
        Wk.reshape(KC, P, MI, P).transpose(1, 2, 0, 3)
    ).astype(bf)
    Wv_b = np.ascontiguousarray(Wv.reshape(KC, P, INNER).transpose(1, 0, 2)).astype(bf)
    Wout_b = np.ascontiguousarray(
        Wout.reshape(KO, P, QD).transpose(1, 0, 2)
    ).astype(bf)
    bout_r = np.ascontiguousarray(bout.reshape(QT, P).T, dtype=np.float32)
    in_maps = []
    for c in range(8):
        b, half = divmod(c, 2)
        xh = x[b, half * NSH : (half + 1) * NSH, :]  # [NSH, QD]
        xr = xh.reshape(NB, FB, KQ, P).transpose(3, 2, 0, 1)  # [P, KQ, NB, FB]
        xT0 = np.ascontiguousarray(xr[:, :, 0, :]).astype(bf)
        xT1 = np.ascontiguousarray(xr[:, :, 1, :]).astype(bf)
        ctxT = np.ascontiguousarray(
            context[b].reshape(M, KC, P).transpose(2, 1, 0)
        ).astype(bf)
        in_maps.append(
            {
                "Wq0": np.ascontiguousarray(Wq_b[:, 0:1, :, :]),
                "Wqr": np.ascontiguousarray(Wq_b[:, 1:MI, :, :]),
                "xT0a": np.ascontiguousarray(xT0[:, 0:KQH, :]),
                "xT0b": np.ascontiguousarray(xT0[:, KQH:KQ, :]),
                "Wk0": np.ascontiguousarray(Wk_b[:, 0:1, :, :]),
                "Wkr": np.ascontiguousarray(Wk_b[:, 1:MI, :, :]),
                "ctxTa": np.ascontiguousarray(ctxT[:, :, 0:MH]),
                "ctxTb": np.ascontiguousarray(ctxT[:, :, MH:M]),
                "Wv": Wv_b,
                "xT1": xT1,
                "Wout": Wout_b,
                "bout": bout_r,
            }
        )
    return in_maps


def gather_out(results):
    out = np.empty((B, N, QD), dtype=np.float32)
    for c in range(8):
        b, half = divmod(c, 2)
        for nb, key in ((0, "out0"), (1, "out1")):
            blk = results[c][key].astype(np.float32)  # [P, QT, FB]
            out[b, half * NSH + nb * FB : half * NSH + (nb + 1) * FB, :] = (
                blk.transpose(2, 1, 0).reshape(FB, QD)
            )
    return out


def kernel(**inputs):
    nc = _get_nc()
    in_maps = make_in_maps(**inputs)
    res = run_bass_kernel_spmd(nc, in_maps, list(range(8)))
    return gather_out(res.results)


if __name__ == "__main__":
    rng = np.random.default_rng(0)
    ins = {
        "x": rng.standard_normal((B, N, QD), dtype=np.float32),
        "context": rng.standard_normal((B, M, CD), dtype=np.float32),
        "Wq": rng.standard_normal((QD, INNER), dtype=np.float32) / 32,
        "Wk": rng.standard_normal((CD, INNER), dtype=np.float32) / 27.7,
        "Wv": rng.standard_normal((CD, INNER), dtype=np.float32) / 27.7,
        "Wout": rng.standard_normal((INNER, QD), dtype=np.float32) / 22.6,
        "bout": rng.standard_normal((QD,), dtype=np.float32) * 0.01,
    }
    out = kernel(**ins)
    print("out", out.shape, out.dtype, np.abs(out).mean())
